# revision 16
# baseline (speedup 1.0000x reference)
"""Trainium2 Bass kernel for AttentionalAggregation-style GNN pooling.

reference math:
    enc  = relu(lane_encoding @ W.T + b)            # [M=400000, 512]
    maxp = segment_max(enc, seg)                    # [N=25000, 512], 16 lanes/group
    avgp = segment_mean(enc, seg)                   # [N=25000, 512]
    out  = concat([maxp, avgp], axis=1)             # [N, 1024]

Strategy (8 NeuronCores, data-parallel over lanes; each core owns whole groups):
  - Host pre-transposes x -> XT [128, M] (bf16) with an "s-major" column
    permutation inside each 2048-lane block: column s*G + g holds lane s of
    group g.  Pooling over a group then becomes a pairwise halving tree over
    CONTIGUOUS slabs, which runs on the Vector engine in 2x bf16 mode
    (599ns per 1024-out TT vs 2193ns for a 1x windowed reduce).
  - Single bf16 matmul per 512-col wave (PSUM f32 accumulate).  The 2e-2
    rel-err budget makes the bf16x3 compensated split unnecessary.
  - ACT drains PSUM with fused relu(u + b) -> bf16 r-tiles (1888ns/2048).
  - DVE runs max and sum trees on the r-tiles, batched across the 4 outdim
    chunks of a block to amortize per-op overhead.  No GPSIMD: its shared
    SBUF port fully serializes against DVE 2-port TT ops (measured).
  - Outputs stay transposed bf16 [512, G]; host converts / divides by 16.
"""
import sys

sys.path.insert(0, "/opt/trn_rl_repo")

import numpy as np
import ml_dtypes

import concourse.bass as bass
import concourse.bacc as bacc
import concourse.tile as tile
from concourse import mybir
from concourse.bass_utils import run_bass_kernel_spmd

N_CORES = 8
IN_DIM = 128
OUT_DIM = 512
N_OBS = 25000
M_LANES = 400000
GS = 16                       # lanes per group
M_C = M_LANES // N_CORES      # 50000 lanes per core
G_C = N_OBS // N_CORES        # 3125 groups per core
N_CHUNK = OUT_DIM // 128      # 4 outdim chunks
BLK = 2048                    # lanes per block (4 psum banks)
G_PAD = G_C + 1               # 3126: even stride for accumulator tiles
# 1-in-RELU_SPLIT_MOD of the (block, chunk) relu drains runs on DVE
# (tensor_scalar) instead of ACT, balancing the two engines.
RELU_SPLIT_MOD = 10**9

MODE = "bf16tree"


def _block_sizes():
    # small prolog blocks prime the pipeline; the ragged remainder runs
    # EARLY so the kernel epilogue is a clean full-size pipelined block
    sizes = [256, 256, 512, 1024]
    rest = M_C - sum(sizes)
    ragged = rest % BLK
    if ragged:
        sizes.append(ragged)
    sizes += [BLK] * (rest // BLK)
    return sizes

_compiled = {}


def _tree(nc, rblk, dst, gb, op, tpool):
    """Halving tree over the 16 s-slabs of rblk [128, 4, 16*gb] -> dst
    [128, 4, gb].  All levels contiguous-slab TT ops (bf16 2x mode)."""
    bf16 = mybir.dt.bfloat16
    cur = rblk
    for lvl, w in enumerate((8 * gb, 4 * gb, 2 * gb, gb)):
        last = w == gb
        nxt = dst if last else tpool.tile([128, N_CHUNK, w], bf16, tag=f"t{lvl}")
        nc.vector.tensor_tensor(
            out=nxt if last else nxt[:, :, 0:w],
            in0=cur[:, :, 0:w],
            in1=cur[:, :, w : 2 * w],
            op=op,
        )
        cur = nxt


def _build(mode: str) -> bass.Bass:
    nc = bacc.Bacc(None, target_bir_lowering=False)
    f32 = mybir.dt.float32
    bf16 = mybir.dt.bfloat16

    xth_d = nc.dram_tensor("xth", [IN_DIM, M_C], bf16, kind="ExternalInput")
    wth_d = nc.dram_tensor("wth", [IN_DIM, OUT_DIM], bf16, kind="ExternalInput")
    bsc_d = nc.dram_tensor("bsc", [128, N_CHUNK], f32, kind="ExternalInput")
    omax_d = nc.dram_tensor("omax", [OUT_DIM, G_C], bf16, kind="ExternalOutput")
    osum_d = nc.dram_tensor("osum", [OUT_DIM, G_C], bf16, kind="ExternalOutput")

    n_blk = (M_C + BLK - 1) // BLK          # 25 (24 full + tail 848)

    with tile.TileContext(nc) as tc:
        with (
            tc.tile_pool(name="singles", bufs=1) as singles,
            tc.tile_pool(name="xin", bufs=4) as xin,
            tc.tile_pool(name="rblk", bufs=3) as rpool,
            tc.tile_pool(name="trees", bufs=2) as tpool,
            tc.tile_pool(name="acc", bufs=1) as accp,
            tc.tile_pool(name="psum", bufs=2, space="PSUM") as psum,
        ):
            wth_sb = singles.tile([IN_DIM, OUT_DIM], bf16)
            nc.sync.dma_start(out=wth_sb, in_=wth_d[:, :])
            bsc_sb = singles.tile([128, N_CHUNK], f32)
            nc.sync.dma_start(out=bsc_sb, in_=bsc_d[:, :])

            # pooled accumulators [128, chunk, G_C] bf16
            maxp_sb = accp.tile([128, N_CHUNK, G_C], bf16)
            sump_sb = accp.tile([128, N_CHUNK, G_C], bf16)

            # prime ACT spline table before the pipeline starts
            warm_sb = singles.tile([128, 2], f32)
            nc.vector.memset(warm_sb, 0.0)
            nc.scalar.activation(
                out=warm_sb, in_=warm_sb,
                func=mybir.ActivationFunctionType.Relu, bias=0.0, scale=1.0,
            )

            # variable block schedule: small first blocks prime the pipeline
            # (DVE idled ~15us waiting for block0's DMA+matmul+relu at BLK=2048)
            sizes = _block_sizes()
            blocks = []
            l0 = 0
            for lb in sizes:
                blocks.append((l0, lb))
                l0 += lb

            # flush after these block indices (finer near the end to shrink
            # the output-DMA tail)
            flush_at = {5, 10, 15, 19, 22, len(blocks) - 2, len(blocks) - 1}
            flush_g = 0

            for ib, (l0, lb) in enumerate(blocks):
                gb = lb // GS
                g0 = l0 // GS

                xt_sb = xin.tile([IN_DIM, BLK], bf16, tag="xt")
                nc.sync.dma_start(out=xt_sb[:, :lb], in_=xth_d[:, l0 : l0 + lb])


                r_sb = rpool.tile([128, N_CHUNK, BLK], bf16, tag="r")

                n_wave = (lb + 511) // 512
                for c in range(N_CHUNK):
                    enc_ps = psum.tile([128, BLK], f32, tag="enc")
                    for w in range(n_wave):
                        w0 = w * 512
                        lw = min(512, lb - w0)
                        nc.tensor.matmul(
                            enc_ps[:, w0 : w0 + lw],
                            wth_sb[:, c * 128 : (c + 1) * 128],
                            xt_sb[:, w0 : w0 + lw],
                            start=True, stop=True,
                        )
                    # r = relu(u + b): fused PSUM->SBUF drain
                    if (ib * N_CHUNK + c) % RELU_SPLIT_MOD == 0:
                        nc.vector.tensor_scalar(
                            out=r_sb[:, c, :lb],
                            in0=enc_ps[:, :lb],
                            scalar1=bsc_sb[:, c : c + 1],
                            scalar2=0.0,
                            op0=mybir.AluOpType.add,
                            op1=mybir.AluOpType.max,
                        )
                    else:
                        nc.scalar.activation(
                            out=r_sb[:, c, :lb],
                            in_=enc_ps[:, :lb],
                            func=mybir.ActivationFunctionType.Relu,
                            bias=bsc_sb[:, c : c + 1],
                            scale=1.0,
                        )

                # pooling trees over the whole block (4 chunks at once)
                r3 = r_sb[:, :, :lb]
                _tree(nc, r3, maxp_sb[:, :, g0 : g0 + gb], gb,
                      mybir.AluOpType.max, tpool)
                _tree(nc, r3, sump_sb[:, :, g0 : g0 + gb], gb,
                      mybir.AluOpType.add, tpool)

                if ib in flush_at:
                    r0, r1 = flush_g, g0 + gb
                    flush_g = r1
                    for c in range(N_CHUNK):
                        nc.sync.dma_start(
                            out=omax_d[c * 128 : (c + 1) * 128, r0:r1],
                            in_=maxp_sb[:, c, r0:r1],
                        )
                        nc.sync.dma_start(
                            out=osum_d[c * 128 : (c + 1) * 128, r0:r1],
                            in_=sump_sb[:, c, r0:r1],
                        )

    nc.compile()
    return nc


def _get_nc(mode: str) -> bass.Bass:
    if mode not in _compiled:
        _compiled[mode] = _build(mode)
    return _compiled[mode]


def _host_prep(lane_encoding, W, b, mode: str):
    """Per-core in_maps.  x is transposed, bf16-cast, and column-permuted to
    s-major inside each 2048-lane block."""
    bf = ml_dtypes.bfloat16
    xT = np.ascontiguousarray(lane_encoding.T).astype(bf)   # [128, M]
    wT = np.ascontiguousarray(W.T).astype(bf)               # [128, 512]
    bsc = np.ascontiguousarray(b.reshape(N_CHUNK, 128).T.astype(np.float32))

    sizes = _block_sizes()

    in_maps = []
    for c in range(N_CORES):
        xc = xT[:, c * M_C : (c + 1) * M_C]
        parts = []
        l0 = 0
        for lb in sizes:
            blkv = xc[:, l0 : l0 + lb].reshape(IN_DIM, lb // GS, GS)
            parts.append(blkv.transpose(0, 2, 1).reshape(IN_DIM, lb))
            l0 += lb
        xs = np.ascontiguousarray(np.concatenate(parts, axis=1))
        in_maps.append({"xth": xs, "wth": wT, "bsc": bsc})
    return in_maps


def _run(lane_encoding, W, b, mode: str, trace: bool = False):
    nc = _get_nc(mode)
    in_maps = _host_prep(lane_encoding, W, b, mode)
    try:
        res = run_bass_kernel_spmd(
            nc, in_maps, core_ids=list(range(N_CORES)), trace=trace
        )
    except Exception:
        # transient NRT_EXEC_UNIT_UNRECOVERABLE wedges; one retry usually works
        res = run_bass_kernel_spmd(
            nc, in_maps, core_ids=list(range(N_CORES)), trace=trace
        )
    out = np.empty((N_OBS, 2 * OUT_DIM), dtype=np.float32)
    for c in range(N_CORES):
        gsl = slice(c * G_C, (c + 1) * G_C)
        out[gsl, :OUT_DIM] = res.results[c]["omax"].T.astype(np.float32)
        out[gsl, OUT_DIM:] = res.results[c]["osum"].T.astype(np.float32) / GS
    return out, res


def kernel(obs_encoding, lane_encoding, same_obs_mask, W, b):
    out, _ = _run(
        np.asarray(lane_encoding, dtype=np.float32),
        np.asarray(W, dtype=np.float32),
        np.asarray(b, dtype=np.float32),
        MODE,
    )
    return out


# revision 17
# speedup vs baseline: 1.0024x; 1.0024x over previous
"""Trainium2 Bass kernel for AttentionalAggregation-style GNN pooling.

reference math:
    enc  = relu(lane_encoding @ W.T + b)            # [M=400000, 512]
    maxp = segment_max(enc, seg)                    # [N=25000, 512], 16 lanes/group
    avgp = segment_mean(enc, seg)                   # [N=25000, 512]
    out  = concat([maxp, avgp], axis=1)             # [N, 1024]

Strategy (8 NeuronCores, data-parallel over lanes; each core owns whole groups):
  - Host pre-transposes x -> XT [128, M] (bf16) with an "s-major" column
    permutation inside each 2048-lane block: column s*G + g holds lane s of
    group g.  Pooling over a group then becomes a pairwise halving tree over
    CONTIGUOUS slabs, which runs on the Vector engine in 2x bf16 mode
    (599ns per 1024-out TT vs 2193ns for a 1x windowed reduce).
  - Single bf16 matmul per 512-col wave (PSUM f32 accumulate).  The 2e-2
    rel-err budget makes the bf16x3 compensated split unnecessary.
  - ACT drains PSUM with fused relu(u + b) -> bf16 r-tiles (1888ns/2048).
  - DVE runs max and sum trees on the r-tiles, batched across the 4 outdim
    chunks of a block to amortize per-op overhead.  No GPSIMD: its shared
    SBUF port fully serializes against DVE 2-port TT ops (measured).
  - Outputs stay transposed bf16 [512, G]; host converts / divides by 16.
"""
import sys

sys.path.insert(0, "/opt/trn_rl_repo")

import numpy as np
import ml_dtypes

import concourse.bass as bass
import concourse.bacc as bacc
import concourse.tile as tile
from concourse import mybir
from concourse.bass_utils import run_bass_kernel_spmd

N_CORES = 8
IN_DIM = 128
OUT_DIM = 512
N_OBS = 25000
M_LANES = 400000
GS = 16                       # lanes per group
M_C = M_LANES // N_CORES      # 50000 lanes per core
G_C = N_OBS // N_CORES        # 3125 groups per core
N_CHUNK = OUT_DIM // 128      # 4 outdim chunks
BLK = 2048                    # lanes per block (4 psum banks)
G_PAD = G_C + 1               # 3126: even stride for accumulator tiles
# 1-in-RELU_SPLIT_MOD of the (block, chunk) relu drains runs on DVE
# (tensor_scalar) instead of ACT, balancing the two engines.
RELU_SPLIT_MOD = 10**9

MODE = "bf16tree"


def _block_sizes():
    # small prolog blocks prime the pipeline; the ragged remainder runs
    # EARLY so the kernel epilogue is a clean full-size pipelined block
    sizes = [512, 512, 1024]
    rest = M_C - sum(sizes)
    ragged = rest % BLK
    if ragged:
        sizes.append(ragged)
    sizes += [BLK] * (rest // BLK)
    return sizes

_compiled = {}


def _tree(nc, rblk, dst, gb, op, tpool):
    """Halving tree over the 16 s-slabs of rblk [128, 4, 16*gb] -> dst
    [128, 4, gb].  All levels contiguous-slab TT ops (bf16 2x mode)."""
    bf16 = mybir.dt.bfloat16
    cur = rblk
    for lvl, w in enumerate((8 * gb, 4 * gb, 2 * gb, gb)):
        last = w == gb
        nxt = dst if last else tpool.tile([128, N_CHUNK, w], bf16, tag=f"t{lvl}")
        nc.vector.tensor_tensor(
            out=nxt if last else nxt[:, :, 0:w],
            in0=cur[:, :, 0:w],
            in1=cur[:, :, w : 2 * w],
            op=op,
        )
        cur = nxt


def _build(mode: str) -> bass.Bass:
    nc = bacc.Bacc(None, target_bir_lowering=False)
    f32 = mybir.dt.float32
    bf16 = mybir.dt.bfloat16

    xth_d = nc.dram_tensor("xth", [IN_DIM, M_C], bf16, kind="ExternalInput")
    wth_d = nc.dram_tensor("wth", [IN_DIM, OUT_DIM], bf16, kind="ExternalInput")
    bsc_d = nc.dram_tensor("bsc", [128, N_CHUNK], f32, kind="ExternalInput")
    omax_d = nc.dram_tensor("omax", [OUT_DIM, G_C], bf16, kind="ExternalOutput")
    osum_d = nc.dram_tensor("osum", [OUT_DIM, G_C], bf16, kind="ExternalOutput")

    n_blk = (M_C + BLK - 1) // BLK          # 25 (24 full + tail 848)

    with tile.TileContext(nc) as tc:
        with (
            tc.tile_pool(name="singles", bufs=1) as singles,
            tc.tile_pool(name="xin", bufs=3) as xin,
            tc.tile_pool(name="rblk", bufs=2) as rpool,
            tc.tile_pool(name="trees", bufs=2) as tpool,
            tc.tile_pool(name="acc", bufs=1) as accp,
            tc.tile_pool(name="psum", bufs=2, space="PSUM") as psum,
        ):
            wth_sb = singles.tile([IN_DIM, OUT_DIM], bf16)
            nc.sync.dma_start(out=wth_sb, in_=wth_d[:, :])
            bsc_sb = singles.tile([128, N_CHUNK], f32)
            nc.sync.dma_start(out=bsc_sb, in_=bsc_d[:, :])

            # pooled accumulators [128, chunk, G_C] bf16
            maxp_sb = accp.tile([128, N_CHUNK, G_C], bf16)
            sump_sb = accp.tile([128, N_CHUNK, G_C], bf16)

            # prime ACT spline table before the pipeline starts
            warm_sb = singles.tile([128, 2], f32)
            nc.vector.memset(warm_sb, 0.0)
            nc.scalar.activation(
                out=warm_sb, in_=warm_sb,
                func=mybir.ActivationFunctionType.Relu, bias=0.0, scale=1.0,
            )

            # variable block schedule: small first blocks prime the pipeline
            # (DVE idled ~15us waiting for block0's DMA+matmul+relu at BLK=2048)
            sizes = _block_sizes()
            blocks = []
            l0 = 0
            for lb in sizes:
                blocks.append((l0, lb))
                l0 += lb

            # flush after these block indices (finer near the end to shrink
            # the output-DMA tail)
            flush_at = {5, 10, 15, 19, 22, len(blocks) - 2, len(blocks) - 1}
            flush_g = 0

            for ib, (l0, lb) in enumerate(blocks):
                gb = lb // GS
                g0 = l0 // GS

                xt_sb = xin.tile([IN_DIM, BLK], bf16, tag="xt")
                nc.sync.dma_start(out=xt_sb[:, :lb], in_=xth_d[:, l0 : l0 + lb])


                r_sb = rpool.tile([128, N_CHUNK, BLK], bf16, tag="r")

                n_wave = (lb + 511) // 512
                for c in range(N_CHUNK):
                    enc_ps = psum.tile([128, BLK], f32, tag="enc")
                    for w in range(n_wave):
                        w0 = w * 512
                        lw = min(512, lb - w0)
                        nc.tensor.matmul(
                            enc_ps[:, w0 : w0 + lw],
                            wth_sb[:, c * 128 : (c + 1) * 128],
                            xt_sb[:, w0 : w0 + lw],
                            start=True, stop=True,
                        )
                    # r = relu(u + b): fused PSUM->SBUF drain
                    if (ib * N_CHUNK + c) % RELU_SPLIT_MOD == 0:
                        nc.vector.tensor_scalar(
                            out=r_sb[:, c, :lb],
                            in0=enc_ps[:, :lb],
                            scalar1=bsc_sb[:, c : c + 1],
                            scalar2=0.0,
                            op0=mybir.AluOpType.add,
                            op1=mybir.AluOpType.max,
                        )
                    else:
                        nc.scalar.activation(
                            out=r_sb[:, c, :lb],
                            in_=enc_ps[:, :lb],
                            func=mybir.ActivationFunctionType.Relu,
                            bias=bsc_sb[:, c : c + 1],
                            scale=1.0,
                        )

                # pooling trees over the whole block (4 chunks at once)
                r3 = r_sb[:, :, :lb]
                _tree(nc, r3, maxp_sb[:, :, g0 : g0 + gb], gb,
                      mybir.AluOpType.max, tpool)
                _tree(nc, r3, sump_sb[:, :, g0 : g0 + gb], gb,
                      mybir.AluOpType.add, tpool)

                if ib in flush_at:
                    r0, r1 = flush_g, g0 + gb
                    flush_g = r1
                    for c in range(N_CHUNK):
                        nc.sync.dma_start(
                            out=omax_d[c * 128 : (c + 1) * 128, r0:r1],
                            in_=maxp_sb[:, c, r0:r1],
                        )
                        nc.sync.dma_start(
                            out=osum_d[c * 128 : (c + 1) * 128, r0:r1],
                            in_=sump_sb[:, c, r0:r1],
                        )

    nc.compile()
    return nc


def _get_nc(mode: str) -> bass.Bass:
    if mode not in _compiled:
        _compiled[mode] = _build(mode)
    return _compiled[mode]


def _host_prep(lane_encoding, W, b, mode: str):
    """Per-core in_maps.  x is transposed, bf16-cast, and column-permuted to
    s-major inside each 2048-lane block."""
    bf = ml_dtypes.bfloat16
    xT = np.ascontiguousarray(lane_encoding.T).astype(bf)   # [128, M]
    wT = np.ascontiguousarray(W.T).astype(bf)               # [128, 512]
    bsc = np.ascontiguousarray(b.reshape(N_CHUNK, 128).T.astype(np.float32))

    sizes = _block_sizes()

    in_maps = []
    for c in range(N_CORES):
        xc = xT[:, c * M_C : (c + 1) * M_C]
        parts = []
        l0 = 0
        for lb in sizes:
            blkv = xc[:, l0 : l0 + lb].reshape(IN_DIM, lb // GS, GS)
            parts.append(blkv.transpose(0, 2, 1).reshape(IN_DIM, lb))
            l0 += lb
        xs = np.ascontiguousarray(np.concatenate(parts, axis=1))
        in_maps.append({"xth": xs, "wth": wT, "bsc": bsc})
    return in_maps


def _run(lane_encoding, W, b, mode: str, trace: bool = False):
    nc = _get_nc(mode)
    in_maps = _host_prep(lane_encoding, W, b, mode)
    try:
        res = run_bass_kernel_spmd(
            nc, in_maps, core_ids=list(range(N_CORES)), trace=trace
        )
    except Exception:
        # transient NRT_EXEC_UNIT_UNRECOVERABLE wedges; one retry usually works
        res = run_bass_kernel_spmd(
            nc, in_maps, core_ids=list(range(N_CORES)), trace=trace
        )
    out = np.empty((N_OBS, 2 * OUT_DIM), dtype=np.float32)
    for c in range(N_CORES):
        gsl = slice(c * G_C, (c + 1) * G_C)
        out[gsl, :OUT_DIM] = res.results[c]["omax"].T.astype(np.float32)
        out[gsl, OUT_DIM:] = res.results[c]["osum"].T.astype(np.float32) / GS
    return out, res


def kernel(obs_encoding, lane_encoding, same_obs_mask, W, b):
    out, _ = _run(
        np.asarray(lane_encoding, dtype=np.float32),
        np.asarray(W, dtype=np.float32),
        np.asarray(b, dtype=np.float32),
        MODE,
    )
    return out


# revision 18
# speedup vs baseline: 1.0036x; 1.0011x over previous
"""Trainium2 Bass kernel for AttentionalAggregation-style GNN pooling.

reference math:
    enc  = relu(lane_encoding @ W.T + b)            # [M=400000, 512]
    maxp = segment_max(enc, seg)                    # [N=25000, 512], 16 lanes/group
    avgp = segment_mean(enc, seg)                   # [N=25000, 512]
    out  = concat([maxp, avgp], axis=1)             # [N, 1024]

Strategy (8 NeuronCores, data-parallel over lanes; each core owns whole groups):
  - Host pre-transposes x -> XT [128, M] (bf16) with an "s-major" column
    permutation inside each 2048-lane block: column s*G + g holds lane s of
    group g.  Pooling over a group then becomes a pairwise halving tree over
    CONTIGUOUS slabs, which runs on the Vector engine in 2x bf16 mode
    (599ns per 1024-out TT vs 2193ns for a 1x windowed reduce).
  - Single bf16 matmul per 512-col wave (PSUM f32 accumulate).  The 2e-2
    rel-err budget makes the bf16x3 compensated split unnecessary.
  - ACT drains PSUM with fused relu(u + b) -> bf16 r-tiles (1888ns/2048).
  - DVE runs max and sum trees on the r-tiles, batched across the 4 outdim
    chunks of a block to amortize per-op overhead.  No GPSIMD: its shared
    SBUF port fully serializes against DVE 2-port TT ops (measured).
  - Outputs stay transposed bf16 [512, G]; host converts / divides by 16.
"""
import sys

sys.path.insert(0, "/opt/trn_rl_repo")

import numpy as np
import ml_dtypes

import concourse.bass as bass
import concourse.bacc as bacc
import concourse.tile as tile
from concourse import mybir
from concourse.bass_utils import run_bass_kernel_spmd

N_CORES = 8
IN_DIM = 128
OUT_DIM = 512
N_OBS = 25000
M_LANES = 400000
GS = 16                       # lanes per group
M_C = M_LANES // N_CORES      # 50000 lanes per core
G_C = N_OBS // N_CORES        # 3125 groups per core
N_CHUNK = OUT_DIM // 128      # 4 outdim chunks
BLK = 2048                    # lanes per block (4 psum banks)
# 1-in-RELU_SPLIT_MOD of the (block, chunk) relu drains runs on DVE
# (tensor_scalar) instead of ACT.  Disabled: the DVE (trees) is the
# saturated engine, so moving drain work onto it only hurts (measured).
RELU_SPLIT_MOD = 10**9

MODE = "bf16tree"


def _block_sizes():
    # small prolog blocks prime the pipeline; the ragged remainder runs
    # EARLY so the kernel epilogue is a clean full-size pipelined block
    sizes = [512, 512, 1024]
    rest = M_C - sum(sizes)
    ragged = rest % BLK
    if ragged:
        sizes.append(ragged)
    sizes += [BLK] * (rest // BLK)
    return sizes

_compiled = {}


def _tree(nc, rblk, dst, gb, op, tpool):
    """Halving tree over the 16 s-slabs of rblk [128, 4, 16*gb] -> dst
    [128, 4, gb].  All levels contiguous-slab TT ops (bf16 2x mode)."""
    bf16 = mybir.dt.bfloat16
    cur = rblk
    for lvl, w in enumerate((8 * gb, 4 * gb, 2 * gb, gb)):
        last = w == gb
        nxt = dst if last else tpool.tile([128, N_CHUNK, w], bf16, tag=f"t{lvl}")
        nc.vector.tensor_tensor(
            out=nxt if last else nxt[:, :, 0:w],
            in0=cur[:, :, 0:w],
            in1=cur[:, :, w : 2 * w],
            op=op,
        )
        cur = nxt


def _build(mode: str) -> bass.Bass:
    nc = bacc.Bacc(None, target_bir_lowering=False)
    f32 = mybir.dt.float32
    bf16 = mybir.dt.bfloat16

    xth_d = nc.dram_tensor("xth", [IN_DIM, M_C], bf16, kind="ExternalInput")
    wth_d = nc.dram_tensor("wth", [IN_DIM, OUT_DIM], bf16, kind="ExternalInput")
    bsc_d = nc.dram_tensor("bsc", [128, N_CHUNK], f32, kind="ExternalInput")
    omax_d = nc.dram_tensor("omax", [OUT_DIM, G_C], bf16, kind="ExternalOutput")
    osum_d = nc.dram_tensor("osum", [OUT_DIM, G_C], bf16, kind="ExternalOutput")

    n_blk = (M_C + BLK - 1) // BLK          # 25 (24 full + tail 848)

    with tile.TileContext(nc) as tc:
        with (
            tc.tile_pool(name="singles", bufs=1) as singles,
            tc.tile_pool(name="xin", bufs=3) as xin,
            tc.tile_pool(name="rblk", bufs=2) as rpool,
            tc.tile_pool(name="trees", bufs=2) as tpool,
            tc.tile_pool(name="acc", bufs=1) as accp,
            tc.tile_pool(name="psum", bufs=2, space="PSUM") as psum,
        ):
            wth_sb = singles.tile([IN_DIM, OUT_DIM], bf16)
            nc.sync.dma_start(out=wth_sb, in_=wth_d[:, :])
            bsc_sb = singles.tile([128, N_CHUNK], f32)
            nc.sync.dma_start(out=bsc_sb, in_=bsc_d[:, :])

            # pooled accumulators [128, chunk, G_C] bf16
            maxp_sb = accp.tile([128, N_CHUNK, G_C], bf16)
            sump_sb = accp.tile([128, N_CHUNK, G_C], bf16)

            # prime ACT spline table before the pipeline starts
            warm_sb = singles.tile([128, 2], f32)
            nc.vector.memset(warm_sb, 0.0)
            nc.scalar.activation(
                out=warm_sb, in_=warm_sb,
                func=mybir.ActivationFunctionType.Relu, bias=0.0, scale=1.0,
            )

            # variable block schedule: small first blocks prime the pipeline
            # (DVE idled ~15us waiting for block0's DMA+matmul+relu at BLK=2048)
            sizes = _block_sizes()
            blocks = []
            l0 = 0
            for lb in sizes:
                blocks.append((l0, lb))
                l0 += lb

            # flush after these block indices (finer near the end to shrink
            # the output-DMA tail)
            flush_at = {5, 10, 15, 19, 22, len(blocks) - 2, len(blocks) - 1}
            flush_g = 0

            for ib, (l0, lb) in enumerate(blocks):
                gb = lb // GS
                g0 = l0 // GS

                xt_sb = xin.tile([IN_DIM, BLK], bf16, tag="xt")
                nc.sync.dma_start(out=xt_sb[:, :lb], in_=xth_d[:, l0 : l0 + lb])


                r_sb = rpool.tile([128, N_CHUNK, BLK], bf16, tag="r")

                n_wave = (lb + 511) // 512
                for c in range(N_CHUNK):
                    enc_ps = psum.tile([128, BLK], f32, tag="enc")
                    for w in range(n_wave):
                        w0 = w * 512
                        lw = min(512, lb - w0)
                        nc.tensor.matmul(
                            enc_ps[:, w0 : w0 + lw],
                            wth_sb[:, c * 128 : (c + 1) * 128],
                            xt_sb[:, w0 : w0 + lw],
                            start=True, stop=True,
                        )
                    # r = relu(u + b): fused PSUM->SBUF drain
                    if (ib * N_CHUNK + c) % RELU_SPLIT_MOD == 0:
                        nc.vector.tensor_scalar(
                            out=r_sb[:, c, :lb],
                            in0=enc_ps[:, :lb],
                            scalar1=bsc_sb[:, c : c + 1],
                            scalar2=0.0,
                            op0=mybir.AluOpType.add,
                            op1=mybir.AluOpType.max,
                        )
                    else:
                        nc.scalar.activation(
                            out=r_sb[:, c, :lb],
                            in_=enc_ps[:, :lb],
                            func=mybir.ActivationFunctionType.Relu,
                            bias=bsc_sb[:, c : c + 1],
                            scale=1.0,
                        )

                # pooling trees over the whole block (4 chunks at once)
                r3 = r_sb[:, :, :lb]
                _tree(nc, r3, maxp_sb[:, :, g0 : g0 + gb], gb,
                      mybir.AluOpType.max, tpool)
                _tree(nc, r3, sump_sb[:, :, g0 : g0 + gb], gb,
                      mybir.AluOpType.add, tpool)

                if ib in flush_at:
                    r0, r1 = flush_g, g0 + gb
                    flush_g = r1
                    for c in range(N_CHUNK):
                        nc.sync.dma_start(
                            out=omax_d[c * 128 : (c + 1) * 128, r0:r1],
                            in_=maxp_sb[:, c, r0:r1],
                        )
                        nc.sync.dma_start(
                            out=osum_d[c * 128 : (c + 1) * 128, r0:r1],
                            in_=sump_sb[:, c, r0:r1],
                        )

    nc.compile()
    return nc


def _get_nc(mode: str) -> bass.Bass:
    if mode not in _compiled:
        _compiled[mode] = _build(mode)
    return _compiled[mode]


def _host_prep(lane_encoding, W, b, mode: str):
    """Per-core in_maps.  x is transposed, bf16-cast, and column-permuted to
    s-major inside each 2048-lane block."""
    bf = ml_dtypes.bfloat16
    xT = np.ascontiguousarray(lane_encoding.T).astype(bf)   # [128, M]
    wT = np.ascontiguousarray(W.T).astype(bf)               # [128, 512]
    bsc = np.ascontiguousarray(b.reshape(N_CHUNK, 128).T.astype(np.float32))

    sizes = _block_sizes()

    in_maps = []
    for c in range(N_CORES):
        xc = xT[:, c * M_C : (c + 1) * M_C]
        parts = []
        l0 = 0
        for lb in sizes:
            blkv = xc[:, l0 : l0 + lb].reshape(IN_DIM, lb // GS, GS)
            parts.append(blkv.transpose(0, 2, 1).reshape(IN_DIM, lb))
            l0 += lb
        xs = np.ascontiguousarray(np.concatenate(parts, axis=1))
        in_maps.append({"xth": xs, "wth": wT, "bsc": bsc})
    return in_maps


def _run(lane_encoding, W, b, mode: str, trace: bool = False):
    nc = _get_nc(mode)
    in_maps = _host_prep(lane_encoding, W, b, mode)
    try:
        res = run_bass_kernel_spmd(
            nc, in_maps, core_ids=list(range(N_CORES)), trace=trace
        )
    except Exception:
        # transient NRT_EXEC_UNIT_UNRECOVERABLE wedges; one retry usually works
        res = run_bass_kernel_spmd(
            nc, in_maps, core_ids=list(range(N_CORES)), trace=trace
        )
    out = np.empty((N_OBS, 2 * OUT_DIM), dtype=np.float32)
    for c in range(N_CORES):
        gsl = slice(c * G_C, (c + 1) * G_C)
        out[gsl, :OUT_DIM] = res.results[c]["omax"].T.astype(np.float32)
        out[gsl, OUT_DIM:] = res.results[c]["osum"].T.astype(np.float32) / GS
    return out, res


def kernel(obs_encoding, lane_encoding, same_obs_mask, W, b):
    out, _ = _run(
        np.asarray(lane_encoding, dtype=np.float32),
        np.asarray(W, dtype=np.float32),
        np.asarray(b, dtype=np.float32),
        MODE,
    )
    return out
